# revision 21
# baseline (speedup 1.0000x reference)
"""Distributed causal multi-head attention block for Trainium2 (8 NeuronCores).

Problem: x[2,2048,1024] -> c_attn(QKV) -> 16-head causal attention -> c_proj.

Sharding (hardcoded): DP=2 on batch x TP=4 on heads. Core c handles
batch b=c//4 and heads 4*(c%4)..4*(c%4)+3. Each core computes Q^T,K^T
(hidden-transposed layout), V (natural layout, with an appended ones
column for softmax row sums), flash-style causal attention with scores
kept transposed [keys, queries] so no on-device transposes are needed,
then normalizes z by the softmax row sums. z^T shards ([256,2048] bf16)
are AllGather'd within each batch group of 4 cores, and each core
computes the c_proj for a 256-wide slice of the output-feature axis
(the w_proj column slice is baked into that core's input), so the final
host-side step is a pure concatenation.

Compute dtype bf16 on the TensorEngine, f32 softmax statistics.
x is pre-transposed/cast on the host (input marshalling) so the device
graph has zero transposes.
"""
import contextlib
import ctypes
import os
import sys
import types

import numpy as np

# ---------------------------------------------------------------- problem dims
B, S, D = 2, 2048, 1024
H, HD = 16, 64
N_CORES = 8
TP = 4                   # cores per batch group (head-parallel)
HPC = H // TP            # heads per core = 4
QCOLS = HPC * HD         # 256 q (and k, v) columns per core
ESL = D // TP            # 256 output-feature columns per core
KC = D // 128            # 8 contraction chunks
NKT = S // 128           # 16 key tiles
NQT = S // 512           # 4 query tiles (512-wide, free dim)
GROUPS = [[0, 1, 2, 3], [4, 5, 6, 7]]


def _install_ntff_shim():
    """Make `antenv.axon_hooks` importable so BASS_TRACE profiling works."""
    if "antenv.axon_hooks" in sys.modules:
        return
    try:
        lib = ctypes.CDLL("/opt/axon/libaxon_pjrt.so")
        lib.axon_start_nrt_profile.argtypes = [ctypes.POINTER(ctypes.c_int64), ctypes.c_size_t]
        lib.axon_start_nrt_profile.restype = ctypes.c_int64
        lib.axon_stop_nrt_profile.argtypes = [ctypes.c_char_p]
        lib.axon_stop_nrt_profile.restype = ctypes.c_int64
    except (OSError, AttributeError):
        lib = None

    @contextlib.contextmanager
    def _hook(output_dir, device_ids):
        import jax
        jax.devices()
        if device_ids:
            ids = (ctypes.c_int64 * len(device_ids))(*device_ids)
            rc = lib.axon_start_nrt_profile(ids, len(device_ids))
        else:
            rc = lib.axon_start_nrt_profile(None, 0)
        if rc != 0:
            raise RuntimeError(f"axon_start_nrt_profile rc={rc}")
        try:
            yield
        finally:
            n = lib.axon_stop_nrt_profile(str(output_dir).encode())
            print(f"profile: {n} file(s) written to {output_dir}", file=sys.stderr)

    mod = types.ModuleType("antenv.axon_hooks")
    mod.get_axon_ntff_profile_hook = lambda: (_hook if lib is not None else None)
    mod.set_axon_ntff_profile_hook = lambda h: None
    sys.modules["antenv.axon_hooks"] = mod


_install_ntff_shim()

import concourse.bacc as bacc
import concourse.mybir as mybir
import concourse.tile as tile
from concourse.bass_utils import run_bass_kernel_spmd

F32 = mybir.dt.float32
BF16 = mybir.dt.bfloat16
NPBF16 = np.dtype(mybir.dt.np(BF16))
EXP = mybir.ActivationFunctionType.Exp
MUL = mybir.AluOpType.mult
ADD = mybir.AluOpType.add


def build_graph():
    nc = bacc.Bacc("TRN2", target_bir_lowering=False, debug=False,
                   enable_asserts=True, num_devices=N_CORES)

    xT_d = nc.dram_tensor("xT", [D, S], BF16, kind="ExternalInput")
    wqk_d = nc.dram_tensor("wqk", [D, 2 * QCOLS], BF16, kind="ExternalInput")
    wv_d = nc.dram_tensor("wv", [D, QCOLS], BF16, kind="ExternalInput")
    wp_d = nc.dram_tensor("wp", [D, ESL], BF16, kind="ExternalInput")
    bqk_d = nc.dram_tensor("bqk", [128, 4], F32, kind="ExternalInput")
    bv_d = nc.dram_tensor("bv", [128, QCOLS], F32, kind="ExternalInput")
    bp_d = nc.dram_tensor("bp", [128, ESL], F32, kind="ExternalInput")
    tri_d = nc.dram_tensor("tri", [128, 128], BF16, kind="ExternalInput")
    ones_d = nc.dram_tensor("ones64", [65, HD], BF16, kind="ExternalInput")
    out_d = nc.dram_tensor("out", [S, ESL], F32, kind="ExternalOutput")

    with tile.TileContext(nc) as tc:
        with (
            tc.tile_pool(name="sb", bufs=1) as sb,
            tc.tile_pool(name="pt", bufs=3) as ptp,
            tc.tile_pool(name="ob", bufs=3) as obp,
            tc.tile_pool(name="rb", bufs=2) as rbp,
            tc.tile_pool(name="psA", bufs=3, space="PSUM") as psA,
            tc.tile_pool(name="psZ", bufs=2, space="PSUM") as psZ,
            tc.tile_pool(name="dram", bufs=2, space="DRAM") as dram,
        ):
            # ---------------- persistent SBUF tensors ----------------
            xT_sb = sb.tile([128, KC * S], BF16, tag="xT")
            wqk_sb = sb.tile([128, KC * 2 * QCOLS], BF16, tag="wqk")
            wv_sb = sb.tile([128, KC * QCOLS], BF16, tag="wv")
            wp_sb = sb.tile([128, KC * ESL], BF16, tag="wp")
            qT_sb = sb.tile([128, 2 * S], BF16, tag="qT")
            kT_sb = sb.tile([128, 2 * S], BF16, tag="kT")
            v_sb = sb.tile([128, NKT * HPC * (HD + 1)], BF16, tag="v")
            zaug_sb = sb.tile([HD, HPC * NQT * 512], BF16, tag="zaug")
            # softmax row sums, packed on matmul-legal partitions {0,32}:
            # (h,qt) -> (row 32*(qt%2), cols h*1024 + 512*(qt//2)), so each
            # head's four sums live in one [65, 1024] block for per-head Ln/Exp.
            r_sum = sb.tile([65, HPC * 1024], F32, tag="rsum")
            r_tmp = sb.tile([65, HPC * 1024], F32, tag="rtmp")
            r_invb = sb.tile([65, HPC * 1024], BF16, tag="rinvb")
            z_sb = sb.tile([128, 2 * S], BF16, tag="z")
            zg_sb = sb.tile([128, KC * S], BF16, tag="zg")
            out_acc = sb.tile([128, NKT * ESL], F32, tag="oacc")
            bqk_sb = sb.tile([128, 4], F32, tag="bqk")
            bv_sb = sb.tile([128, QCOLS], F32, tag="bv")
            bp_sb = sb.tile([128, ESL], F32, tag="bp")
            tri_sb = sb.tile([128, 128], BF16, tag="tri")
            ones_sb = sb.tile([65, HD], BF16, tag="ones")

            # ------------- input DMAs (one batched DMA per tensor) -------------
            def load2(dst_sb, src_d, fs):
                nc.sync.dma_start(
                    out=dst_sb[:].rearrange("p (k s) -> p k s", k=KC),
                    in_=src_d[:, :].rearrange("(k p) s -> p k s", p=128))

            load2(wqk_sb, wqk_d, 512)
            load2(wv_sb, wv_d, QCOLS)
            load2(xT_sb, xT_d, S)
            load2(wp_sb, wp_d, ESL)
            nc.sync.dma_start(out=bqk_sb[:], in_=bqk_d[:])
            nc.sync.dma_start(out=bv_sb[:], in_=bv_d[:])
            nc.sync.dma_start(out=bp_sb[:], in_=bp_d[:])
            nc.sync.dma_start(out=tri_sb[:], in_=tri_d[:])
            nc.sync.dma_start(out=ones_sb[:], in_=ones_d[:])

            # ones columns of V_aug (overwritten below except col 64 per head)
            nc.vector.memset(v_sb[:], 1.0)

            # ---------------- phase A + B interleaved ----------------
            # Emit Q/K for head-pair 0 and all of V first, then attention for
            # pair 0; Q/K for pair 1 is emitted next so its matmuls fill the
            # PE gaps while pair-0 attention waits on the ScalarEngine exps.
            def qk_proj(mc):          # mc 0,1 -> Q head pairs; 2,3 -> K
                for nt in range(NQT):
                    ps = psA.tile([128, 512], F32, tag="m")
                    for k in range(KC):
                        nc.tensor.matmul(
                            ps[:],
                            lhsT=wqk_sb[:, k * 512 + mc * 128: k * 512 + (mc + 1) * 128],
                            rhs=xT_sb[:, k * S + nt * 512: k * S + (nt + 1) * 512],
                            start=(k == 0), stop=(k == KC - 1))
                    dst = qT_sb if mc < 2 else kT_sb
                    c2 = mc % 2
                    nc.vector.tensor_scalar_add(
                        dst[:, c2 * S + nt * 512: c2 * S + (nt + 1) * 512],
                        ps[:], bqk_sb[:, mc:mc + 1])

            def v_proj(t):            # V natural orientation, token tile t
                psv = psA.tile([128, QCOLS], F32, tag="m")
                for k in range(KC):
                    nc.tensor.matmul(
                        psv[:],
                        lhsT=xT_sb[:, k * S + t * 128: k * S + (t + 1) * 128],
                        rhs=wv_sb[:, k * QCOLS:(k + 1) * QCOLS],
                        start=(k == 0), stop=(k == KC - 1))
                vdst = v_sb[:].rearrange(
                    "p (t h e) -> p t h e", t=NKT, e=HD + 1)[:, t, :, 0:HD]
                nc.vector.tensor_tensor(
                    vdst,
                    psv[:].rearrange("p (h d) -> p h d", h=HPC),
                    bv_sb[:].rearrange("p (h d) -> p h d", h=HPC),
                    ADD)

            def attention_pair(hp):
                # Both heads of the pair run together: their K=64 score
                # matmuls occupy PE row-groups 0-63 / 64-127 concurrently and
                # write the two bank-aligned halves of one psum tile, so one
                # exp call covers both heads.
                for qt in range(NQT):
                    q0 = qt * 512
                    n_kt = 4 * qt + 4
                    zaug = [psZ.tile([HD + 1, 512], F32, tag="z", name=f"zaug{hh_}")
                            for hh_ in range(2)]
                    for kt in range(n_kt):
                        k0 = kt * 128
                        qstart = max(q0, k0)
                        w = q0 + 512 - qstart
                        st = psA.tile([128, 1024], F32, tag="m")
                        pT = ptp.tile([128, 1024], BF16, tag="pT")
                        for hh in range(2):
                            ho = hh * HD
                            nc.tensor.matmul(
                                st[:, hh * 512: hh * 512 + w],
                                lhsT=kT_sb[ho:ho + HD, hp * S + k0: hp * S + k0 + 128],
                                rhs=qT_sb[ho:ho + HD, hp * S + qstart: hp * S + qstart + w],
                                start=True, stop=True)
                        ext = 512 + w
                        nc.scalar.activation(pT[:, 0:ext], st[:, 0:ext], EXP, scale=0.125)
                        if k0 >= q0:   # diagonal tile: causal triangle mask
                            for hh in range(2):
                                nc.vector.tensor_tensor(
                                    pT[:, hh * 512: hh * 512 + 128],
                                    pT[:, hh * 512: hh * 512 + 128],
                                    tri_sb[:], MUL)
                        vcol = kt * HPC * (HD + 1) + 2 * hp * (HD + 1)
                        for hh in range(2):
                            nc.tensor.matmul(
                                zaug[hh][:, qstart - q0: 512],
                                lhsT=v_sb[:, vcol + hh * (HD + 1): vcol + (hh + 1) * (HD + 1)],
                                rhs=pT[:, hh * 512: hh * 512 + w],
                                start=(kt == 0), stop=(kt == n_kt - 1))
                    for hh in range(2):
                        h = 2 * hp + hh
                        idx = h * NQT + qt
                        nc.vector.tensor_copy(zaug_sb[:, idx * 512:(idx + 1) * 512],
                                              zaug[hh][0:HD, :])
                        ro, co = r_slot(hp, hh, qt)
                        nc.vector.tensor_copy(r_sum[ro:ro + 1, co:co + 512],
                                              zaug[hh][HD:HD + 1, :])
                    if qt >= 2:
                        normalize_and_gather(hp, qt - 2)

            def r_slot(hp, hh, qt):
                # r_sum slot layout, matmul-legal base partitions only.
                # Tokens 0:1536 (qt 0..2) of pair hp live in block 2*hp:
                # rows {0,32,64} by qt, cols {0,512} by head. The last token
                # chunk (qt 3) gets its own block so its AllGather can fire
                # the moment qt 3 finishes.
                if qt == 3:
                    return 32 * hh, (2 * hp + 1) * 1024
                return 32 * qt, 2 * hp * 1024 + 512 * hh

            def normalize_and_gather(hp, th):
                # Normalize z of pair hp for token chunk th (0 -> tokens
                # 0:1536, 1 -> 1536:2048) and AllGather it across the group.
                # 1/r via exp(-ln(r)); Ln and Exp share one ACT table set and
                # the DVE reciprocal costs 8 cycles per free element.
                tok0, tokw = (0, 1536) if th == 0 else (1536, 512)
                qts = (0, 1, 2) if th == 0 else (3,)
                blk = (2 * hp + th) * 1024
                blkw = 1024 if th == 0 else 512
                nc.scalar.activation(r_tmp[:, blk:blk + blkw], r_sum[:, blk:blk + blkw],
                                     mybir.ActivationFunctionType.Ln)
                nc.scalar.activation(r_invb[:, blk:blk + blkw], r_tmp[:, blk:blk + blkw],
                                     EXP, scale=-1.0)
                for hh in range(2):
                    h = 2 * hp + hh
                    ho = hh * HD
                    for qt in qts:
                        idx = h * NQT + qt
                        ro, co = r_slot(hp, hh, qt)
                        rbc = psZ.tile([HD, 512], F32, tag="z")
                        nc.tensor.matmul(rbc[:], lhsT=ones_sb[ro:ro + 1, :],
                                         rhs=r_invb[ro:ro + 1, co:co + 512],
                                         start=True, stop=True)
                        rbc_sb = rbp.tile([HD, 512], BF16, tag="rb")
                        nc.vector.tensor_copy(rbc_sb[:], rbc[:])
                        nc.vector.tensor_tensor(
                            z_sb[ho:ho + HD, hp * S + qt * 512: hp * S + (qt + 1) * 512],
                            zaug_sb[:, idx * 512:(idx + 1) * 512], rbc_sb[:], MUL)
                # AllGather tokens [tok0, tok0+tokw) of this head-pair's z.
                # zg_sb chunk k=4*hp+j holds heads (4j+2hp, 4j+2hp+1) x 64 dims
                # of the gathered group (w_proj rows permuted host-side).
                zd = dram.tile([128, tokw], BF16, tag="zd")
                zgd = dram.tile([TP * 128, tokw], BF16, tag="zgd")
                nc.sync.dma_start(
                    out=zd[:], in_=z_sb[:, hp * S + tok0: hp * S + tok0 + tokw])
                nc.gpsimd.collective_compute(
                    "AllGather", mybir.AluOpType.bypass, replica_groups=GROUPS,
                    ins=[zd.opt()], outs=[zgd.opt()])
                for j in range(TP):
                    nc.sync.dma_start(
                        out=zg_sb[:, (4 * hp + j) * S + tok0:
                                  (4 * hp + j) * S + tok0 + tokw],
                        in_=zgd[j * 128:(j + 1) * 128, :])

            qk_proj(0)                    # Q heads 0,1
            qk_proj(2)                    # K heads 0,1
            for t in range(NKT):
                v_proj(t)
            attention_pair(0)
            qk_proj(1)                    # Q heads 2,3 (fills pair-0 ACT gaps)
            qk_proj(3)                    # K heads 2,3
            attention_pair(1)

            # ------- phase D: c_proj (output-feature slice), three passes ------
            # pass 1 over zg chunks 0..3 (pair-0 gathers) and pass 2a over
            # chunks 4..7 tokens 0:1536 run while later AllGathers are still
            # in flight; pass 2b (tokens 1536:2048) tails the final gather.
            for mt in range(NKT):
                po = psA.tile([128, ESL], F32, tag="m")
                for k in range(KC // 2):
                    nc.tensor.matmul(
                        po[:],
                        lhsT=zg_sb[:, k * S + mt * 128: k * S + (mt + 1) * 128],
                        rhs=wp_sb[:, k * ESL:(k + 1) * ESL],
                        start=(k == 0), stop=(k == KC // 2 - 1))
                nc.vector.tensor_tensor(out_acc[:, mt * ESL:(mt + 1) * ESL],
                                        po[:], bp_sb[:], ADD)
            for mt in range(NKT):
                po = psA.tile([128, ESL], F32, tag="m")
                for k in range(KC // 2, KC):
                    nc.tensor.matmul(
                        po[:],
                        lhsT=zg_sb[:, k * S + mt * 128: k * S + (mt + 1) * 128],
                        rhs=wp_sb[:, k * ESL:(k + 1) * ESL],
                        start=(k == KC // 2), stop=(k == KC - 1))
                o_sb = obp.tile([128, ESL], F32, tag="o")
                nc.vector.tensor_tensor(o_sb[:], po[:],
                                        out_acc[:, mt * ESL:(mt + 1) * ESL], ADD)
                nc.sync.dma_start(out=out_d[mt * 128:(mt + 1) * 128, :], in_=o_sb[:])


    nc.compile()
    return nc



_NC = None


def _get_nc():
    global _NC
    if _NC is None:
        _NC = build_graph()
    return _NC


def _make_in_maps(x, w_attn, b_attn, w_proj, b_proj):
    x = np.asarray(x, dtype=np.float32)
    w_attn = np.asarray(w_attn, dtype=np.float32)
    b_attn = np.asarray(b_attn, dtype=np.float32)
    w_proj = np.asarray(w_proj, dtype=np.float32)
    b_proj = np.asarray(b_proj, dtype=np.float32)

    tri = np.triu(np.ones((128, 128), np.float32)).astype(NPBF16)  # tri[k,j]=1 iff j>=k
    ones64 = np.ones((65, HD), np.float32).astype(NPBF16)
    xT = [np.ascontiguousarray(x[b].T).astype(NPBF16) for b in range(B)]

    in_maps = []
    for c in range(N_CORES):
        b, hg = c // TP, c % TP
        qs, ks, vs = hg * QCOLS, D + hg * QCOLS, 2 * D + hg * QCOLS
        es = (c % TP) * ESL
        wqk = np.concatenate(
            [w_attn[:, qs:qs + QCOLS], w_attn[:, ks:ks + QCOLS]], axis=1
        ).astype(NPBF16)
        wv = np.ascontiguousarray(w_attn[:, vs:vs + QCOLS]).astype(NPBF16)
        # zg_sb chunk k=4*hp+j holds (heads 4j+2hp, 4j+2hp+1) x 64 dims;
        # permute w_proj rows to match the gathered layout.
        perm = np.empty(D, np.int64)
        for k in range(KC):
            hp_, j = k // TP, k % TP
            for p in range(128):
                perm[k * 128 + p] = (4 * j + 2 * hp_ + p // HD) * HD + p % HD
        wp = np.ascontiguousarray(w_proj[perm][:, es:es + ESL]).astype(NPBF16)
        bqk = np.stack([b_attn[qs:qs + 128], b_attn[qs + 128:qs + QCOLS],
                        b_attn[ks:ks + 128], b_attn[ks + 128:ks + QCOLS]],
                       axis=1).astype(np.float32)
        bv = np.ascontiguousarray(
            np.broadcast_to(b_attn[vs:vs + QCOLS], (128, QCOLS))).astype(np.float32)
        bp = np.ascontiguousarray(
            np.broadcast_to(b_proj[es:es + ESL], (128, ESL))).astype(np.float32)
        in_maps.append({
            "xT": xT[b], "wqk": wqk, "wv": wv, "wp": wp,
            "bqk": bqk, "bv": bv, "bp": bp, "tri": tri, "ones64": ones64,
        })
    return in_maps


def kernel(x, w_attn, b_attn, w_proj, b_proj):
    nc = _get_nc()
    in_maps = _make_in_maps(x, w_attn, b_attn, w_proj, b_proj)
    res = run_bass_kernel_spmd(nc, in_maps, core_ids=list(range(N_CORES)),
                               trace=bool(os.environ.get("BASS_TRACE")))
    if res.exec_time_ns is not None:
        print(f"HW exec time: {res.exec_time_ns} ns")
    out = np.empty((B, S, D), np.float32)
    for c in range(N_CORES):
        b, es = c // TP, (c % TP) * ESL
        out[b, :, es:es + ESL] = res.results[c]["out"]
    return out


# revision 22
# speedup vs baseline: 1.0099x; 1.0099x over previous
"""Distributed causal multi-head attention block for Trainium2 (8 NeuronCores).

Problem: x[2,2048,1024] -> c_attn(QKV) -> 16-head causal attention -> c_proj.

Sharding (hardcoded): DP=2 on batch x TP=4 on heads. Core c handles
batch b=c//4 and heads 4*(c%4)..4*(c%4)+3. Each core computes Q^T,K^T
(hidden-transposed layout), V (natural layout, with an appended ones
column for softmax row sums), flash-style causal attention with scores
kept transposed [keys, queries] so no on-device transposes are needed,
then normalizes z by the softmax row sums. z^T shards ([256,2048] bf16)
are AllGather'd within each batch group of 4 cores, and each core
computes the c_proj for a 256-wide slice of the output-feature axis
(the w_proj column slice is baked into that core's input), so the final
host-side step is a pure concatenation.

Compute dtype bf16 on the TensorEngine, f32 softmax statistics.
x is pre-transposed/cast on the host (input marshalling) so the device
graph has zero transposes.
"""
import contextlib
import ctypes
import os
import sys
import types

import numpy as np

# ---------------------------------------------------------------- problem dims
B, S, D = 2, 2048, 1024
H, HD = 16, 64
N_CORES = 8
TP = 4                   # cores per batch group (head-parallel)
HPC = H // TP            # heads per core = 4
QCOLS = HPC * HD         # 256 q (and k, v) columns per core
ESL = D // TP            # 256 output-feature columns per core
KC = D // 128            # 8 contraction chunks
NKT = S // 128           # 16 key tiles
NQT = S // 512           # 4 query tiles (512-wide, free dim)
GROUPS = [[0, 1, 2, 3], [4, 5, 6, 7]]


def _install_ntff_shim():
    """Make `antenv.axon_hooks` importable so BASS_TRACE profiling works."""
    if "antenv.axon_hooks" in sys.modules:
        return
    try:
        lib = ctypes.CDLL("/opt/axon/libaxon_pjrt.so")
        lib.axon_start_nrt_profile.argtypes = [ctypes.POINTER(ctypes.c_int64), ctypes.c_size_t]
        lib.axon_start_nrt_profile.restype = ctypes.c_int64
        lib.axon_stop_nrt_profile.argtypes = [ctypes.c_char_p]
        lib.axon_stop_nrt_profile.restype = ctypes.c_int64
    except (OSError, AttributeError):
        lib = None

    @contextlib.contextmanager
    def _hook(output_dir, device_ids):
        import jax
        jax.devices()
        if device_ids:
            ids = (ctypes.c_int64 * len(device_ids))(*device_ids)
            rc = lib.axon_start_nrt_profile(ids, len(device_ids))
        else:
            rc = lib.axon_start_nrt_profile(None, 0)
        if rc != 0:
            raise RuntimeError(f"axon_start_nrt_profile rc={rc}")
        try:
            yield
        finally:
            n = lib.axon_stop_nrt_profile(str(output_dir).encode())
            print(f"profile: {n} file(s) written to {output_dir}", file=sys.stderr)

    mod = types.ModuleType("antenv.axon_hooks")
    mod.get_axon_ntff_profile_hook = lambda: (_hook if lib is not None else None)
    mod.set_axon_ntff_profile_hook = lambda h: None
    sys.modules["antenv.axon_hooks"] = mod


_install_ntff_shim()

import concourse.bacc as bacc
import concourse.mybir as mybir
import concourse.tile as tile
from concourse.bass_utils import run_bass_kernel_spmd

F32 = mybir.dt.float32
BF16 = mybir.dt.bfloat16
NPBF16 = np.dtype(mybir.dt.np(BF16))
EXP = mybir.ActivationFunctionType.Exp
MUL = mybir.AluOpType.mult
ADD = mybir.AluOpType.add


def build_graph():
    nc = bacc.Bacc("TRN2", target_bir_lowering=False, debug=False,
                   enable_asserts=True, num_devices=N_CORES)

    xT_d = nc.dram_tensor("xT", [D, S], BF16, kind="ExternalInput")
    wqk_d = nc.dram_tensor("wqk", [D, 2 * QCOLS], BF16, kind="ExternalInput")
    wv_d = nc.dram_tensor("wv", [D, QCOLS], BF16, kind="ExternalInput")
    wp_d = nc.dram_tensor("wp", [D, ESL], BF16, kind="ExternalInput")
    bqk_d = nc.dram_tensor("bqk", [128, 4], F32, kind="ExternalInput")
    bv_d = nc.dram_tensor("bv", [128, QCOLS], F32, kind="ExternalInput")
    bp_d = nc.dram_tensor("bp", [128, ESL], F32, kind="ExternalInput")
    tri_d = nc.dram_tensor("tri", [128, 128], BF16, kind="ExternalInput")
    ones_d = nc.dram_tensor("ones64", [65, HD], BF16, kind="ExternalInput")
    out_d = nc.dram_tensor("out", [S, ESL], F32, kind="ExternalOutput")

    with tile.TileContext(nc) as tc:
        with (
            tc.tile_pool(name="sb", bufs=1) as sb,
            tc.tile_pool(name="pt", bufs=3) as ptp,
            tc.tile_pool(name="ob", bufs=3) as obp,
            tc.tile_pool(name="rb", bufs=2) as rbp,
            tc.tile_pool(name="psA", bufs=3, space="PSUM") as psA,
            tc.tile_pool(name="psZ", bufs=2, space="PSUM") as psZ,
            tc.tile_pool(name="dram", bufs=2, space="DRAM") as dram,
        ):
            # ---------------- persistent SBUF tensors ----------------
            xT_sb = sb.tile([128, KC * S], BF16, tag="xT")
            wqk_sb = sb.tile([128, KC * 2 * QCOLS], BF16, tag="wqk")
            wv_sb = sb.tile([128, KC * QCOLS], BF16, tag="wv")
            wp_sb = sb.tile([128, KC * ESL], BF16, tag="wp")
            qT_sb = sb.tile([128, 2 * S], BF16, tag="qT")
            kT_sb = sb.tile([128, 2 * S], BF16, tag="kT")
            v_sb = sb.tile([128, NKT * HPC * (HD + 1)], BF16, tag="v")
            zaug_sb = sb.tile([HD, HPC * NQT * 512], BF16, tag="zaug")
            # softmax row sums, packed on matmul-legal partitions {0,32}:
            # (h,qt) -> (row 32*(qt%2), cols h*1024 + 512*(qt//2)), so each
            # head's four sums live in one [65, 1024] block for per-head Ln/Exp.
            r_sum = sb.tile([65, HPC * 1024], F32, tag="rsum")
            r_tmp = sb.tile([65, HPC * 1024], F32, tag="rtmp")
            r_invb = sb.tile([65, HPC * 1024], BF16, tag="rinvb")
            z_sb = sb.tile([128, 2 * S], BF16, tag="z")
            zg_sb = sb.tile([128, KC * S], BF16, tag="zg")
            out_acc = sb.tile([128, NKT * ESL], F32, tag="oacc")
            bqk_sb = sb.tile([128, 4], F32, tag="bqk")
            bv_sb = sb.tile([128, QCOLS], F32, tag="bv")
            bp_sb = sb.tile([128, ESL], F32, tag="bp")
            tri_sb = sb.tile([128, 128], BF16, tag="tri")
            ones_sb = sb.tile([65, HD], BF16, tag="ones")

            # ------------- input DMAs (one batched DMA per tensor) -------------
            def load2(dst_sb, src_d, fs):
                nc.sync.dma_start(
                    out=dst_sb[:].rearrange("p (k s) -> p k s", k=KC),
                    in_=src_d[:, :].rearrange("(k p) s -> p k s", p=128))

            load2(wqk_sb, wqk_d, 512)
            load2(wv_sb, wv_d, QCOLS)
            load2(xT_sb, xT_d, S)
            load2(wp_sb, wp_d, ESL)
            nc.sync.dma_start(out=bqk_sb[:], in_=bqk_d[:])
            nc.sync.dma_start(out=bv_sb[:], in_=bv_d[:])
            nc.sync.dma_start(out=bp_sb[:], in_=bp_d[:])
            nc.sync.dma_start(out=tri_sb[:], in_=tri_d[:])
            nc.sync.dma_start(out=ones_sb[:], in_=ones_d[:])

            # ones columns of V_aug (overwritten below except col 64 per head)
            nc.vector.memset(v_sb[:], 1.0)

            # ---------------- phase A + B interleaved ----------------
            # Emit Q/K for head-pair 0 and all of V first, then attention for
            # pair 0; Q/K for pair 1 is emitted next so its matmuls fill the
            # PE gaps while pair-0 attention waits on the ScalarEngine exps.
            def qk_proj(mc):          # mc 0,1 -> Q head pairs; 2,3 -> K
                for nt in range(NQT):
                    ps = psA.tile([128, 512], F32, tag="m")
                    for k in range(KC):
                        nc.tensor.matmul(
                            ps[:],
                            lhsT=wqk_sb[:, k * 512 + mc * 128: k * 512 + (mc + 1) * 128],
                            rhs=xT_sb[:, k * S + nt * 512: k * S + (nt + 1) * 512],
                            start=(k == 0), stop=(k == KC - 1))
                    dst = qT_sb if mc < 2 else kT_sb
                    c2 = mc % 2
                    nc.vector.tensor_scalar_add(
                        dst[:, c2 * S + nt * 512: c2 * S + (nt + 1) * 512],
                        ps[:], bqk_sb[:, mc:mc + 1])

            def v_proj(t):            # V natural orientation, token tile t
                psv = psA.tile([128, QCOLS], F32, tag="m")
                for k in range(KC):
                    nc.tensor.matmul(
                        psv[:],
                        lhsT=xT_sb[:, k * S + t * 128: k * S + (t + 1) * 128],
                        rhs=wv_sb[:, k * QCOLS:(k + 1) * QCOLS],
                        start=(k == 0), stop=(k == KC - 1))
                vdst = v_sb[:].rearrange(
                    "p (t h e) -> p t h e", t=NKT, e=HD + 1)[:, t, :, 0:HD]
                nc.vector.tensor_tensor(
                    vdst,
                    psv[:].rearrange("p (h d) -> p h d", h=HPC),
                    bv_sb[:].rearrange("p (h d) -> p h d", h=HPC),
                    ADD)

            def attention_pair(hp):
                # Both heads of the pair run together: their K=64 score
                # matmuls occupy PE row-groups 0-63 / 64-127 concurrently and
                # write the two bank-aligned halves of one psum tile, so one
                # exp call covers both heads.
                for qt in range(NQT):
                    q0 = qt * 512
                    n_kt = 4 * qt + 4
                    zaug = [psZ.tile([HD + 1, 512], F32, tag="z", name=f"zaug{hh_}")
                            for hh_ in range(2)]
                    for kt in range(n_kt):
                        k0 = kt * 128
                        qstart = max(q0, k0)
                        w = q0 + 512 - qstart
                        st = psA.tile([128, 1024], F32, tag="m")
                        pT = ptp.tile([128, 1024], BF16, tag="pT")
                        for hh in range(2):
                            ho = hh * HD
                            nc.tensor.matmul(
                                st[:, hh * 512: hh * 512 + w],
                                lhsT=kT_sb[ho:ho + HD, hp * S + k0: hp * S + k0 + 128],
                                rhs=qT_sb[ho:ho + HD, hp * S + qstart: hp * S + qstart + w],
                                start=True, stop=True)
                        ext = 512 + w
                        nc.scalar.activation(pT[:, 0:ext], st[:, 0:ext], EXP, scale=0.125)
                        if k0 >= q0:   # diagonal tile: causal triangle mask
                            for hh in range(2):
                                nc.vector.tensor_tensor(
                                    pT[:, hh * 512: hh * 512 + 128],
                                    pT[:, hh * 512: hh * 512 + 128],
                                    tri_sb[:], MUL)
                        vcol = kt * HPC * (HD + 1) + 2 * hp * (HD + 1)
                        for hh in range(2):
                            nc.tensor.matmul(
                                zaug[hh][:, qstart - q0: 512],
                                lhsT=v_sb[:, vcol + hh * (HD + 1): vcol + (hh + 1) * (HD + 1)],
                                rhs=pT[:, hh * 512: hh * 512 + w],
                                start=(kt == 0), stop=(kt == n_kt - 1))
                    for hh in range(2):
                        h = 2 * hp + hh
                        idx = h * NQT + qt
                        nc.vector.tensor_copy(zaug_sb[:, idx * 512:(idx + 1) * 512],
                                              zaug[hh][0:HD, :])
                        ro, co = r_slot(hp, hh, qt)
                        nc.vector.tensor_copy(r_sum[ro:ro + 1, co:co + 512],
                                              zaug[hh][HD:HD + 1, :])
                    normalize_and_gather(hp, qt)

            def r_slot(hp, hh, qt):
                # one [65,512] r_sum block per (pair, query tile); rows {0,32}
                # pick the head within the pair (matmul-legal base partitions).
                return 32 * hh, (hp * NQT + qt) * 512

            def normalize_and_gather(hp, qt):
                # Normalize z of pair hp, query tile qt, then AllGather those
                # 512 tokens across the batch group. Small (128kB) gathers sit
                # near the collective floor, so per-tile gathers overlap the
                # remaining attention work and leave only a tiny final gather.
                # 1/r via exp(-ln(r)); Ln and Exp share one ACT table set and
                # the DVE reciprocal costs 8 cycles per free element.
                blk = (hp * NQT + qt) * 512
                nc.scalar.activation(r_tmp[:, blk:blk + 512], r_sum[:, blk:blk + 512],
                                     mybir.ActivationFunctionType.Ln)
                nc.scalar.activation(r_invb[:, blk:blk + 512], r_tmp[:, blk:blk + 512],
                                     EXP, scale=-1.0)
                for hh in range(2):
                    h = 2 * hp + hh
                    ho = hh * HD
                    idx = h * NQT + qt
                    ro, co = r_slot(hp, hh, qt)
                    rbc = psZ.tile([HD, 512], F32, tag="z")
                    nc.tensor.matmul(rbc[:], lhsT=ones_sb[ro:ro + 1, :],
                                     rhs=r_invb[ro:ro + 1, co:co + 512],
                                     start=True, stop=True)
                    rbc_sb = rbp.tile([HD, 512], BF16, tag="rb")
                    nc.vector.tensor_copy(rbc_sb[:], rbc[:])
                    nc.vector.tensor_tensor(
                        z_sb[ho:ho + HD, hp * S + qt * 512: hp * S + (qt + 1) * 512],
                        zaug_sb[:, idx * 512:(idx + 1) * 512], rbc_sb[:], MUL)
                # zg_sb chunk k=4*hp+j holds heads (4j+2hp, 4j+2hp+1) x 64 dims
                # of the gathered group (w_proj rows permuted host-side).
                zd = dram.tile([128, 512], BF16, tag="zd")
                zgd = dram.tile([TP * 128, 512], BF16, tag="zgd")
                nc.sync.dma_start(
                    out=zd[:], in_=z_sb[:, hp * S + qt * 512: hp * S + (qt + 1) * 512])
                nc.gpsimd.collective_compute(
                    "AllGather", mybir.AluOpType.bypass, replica_groups=GROUPS,
                    ins=[zd.opt()], outs=[zgd.opt()])
                for j in range(TP):
                    nc.sync.dma_start(
                        out=zg_sb[:, (4 * hp + j) * S + qt * 512:
                                  (4 * hp + j) * S + (qt + 1) * 512],
                        in_=zgd[j * 128:(j + 1) * 128, :])

            qk_proj(0)                    # Q heads 0,1
            qk_proj(2)                    # K heads 0,1
            for t in range(NKT):
                v_proj(t)
            attention_pair(0)
            qk_proj(1)                    # Q heads 2,3 (fills pair-0 ACT gaps)
            qk_proj(3)                    # K heads 2,3
            attention_pair(1)

            # ------- phase D: c_proj (output-feature slice), three passes ------
            # pass 1 over zg chunks 0..3 (pair-0 gathers) and pass 2a over
            # chunks 4..7 tokens 0:1536 run while later AllGathers are still
            # in flight; pass 2b (tokens 1536:2048) tails the final gather.
            for mt in range(NKT):
                po = psA.tile([128, ESL], F32, tag="m")
                for k in range(KC // 2):
                    nc.tensor.matmul(
                        po[:],
                        lhsT=zg_sb[:, k * S + mt * 128: k * S + (mt + 1) * 128],
                        rhs=wp_sb[:, k * ESL:(k + 1) * ESL],
                        start=(k == 0), stop=(k == KC // 2 - 1))
                nc.vector.tensor_tensor(out_acc[:, mt * ESL:(mt + 1) * ESL],
                                        po[:], bp_sb[:], ADD)
            for mt in range(NKT):
                po = psA.tile([128, ESL], F32, tag="m")
                for k in range(KC // 2, KC):
                    nc.tensor.matmul(
                        po[:],
                        lhsT=zg_sb[:, k * S + mt * 128: k * S + (mt + 1) * 128],
                        rhs=wp_sb[:, k * ESL:(k + 1) * ESL],
                        start=(k == KC // 2), stop=(k == KC - 1))
                o_sb = obp.tile([128, ESL], F32, tag="o")
                nc.vector.tensor_tensor(o_sb[:], po[:],
                                        out_acc[:, mt * ESL:(mt + 1) * ESL], ADD)
                nc.sync.dma_start(out=out_d[mt * 128:(mt + 1) * 128, :], in_=o_sb[:])


    nc.compile()
    return nc



_NC = None


def _get_nc():
    global _NC
    if _NC is None:
        _NC = build_graph()
    return _NC


def _make_in_maps(x, w_attn, b_attn, w_proj, b_proj):
    x = np.asarray(x, dtype=np.float32)
    w_attn = np.asarray(w_attn, dtype=np.float32)
    b_attn = np.asarray(b_attn, dtype=np.float32)
    w_proj = np.asarray(w_proj, dtype=np.float32)
    b_proj = np.asarray(b_proj, dtype=np.float32)

    tri = np.triu(np.ones((128, 128), np.float32)).astype(NPBF16)  # tri[k,j]=1 iff j>=k
    ones64 = np.ones((65, HD), np.float32).astype(NPBF16)
    xT = [np.ascontiguousarray(x[b].T).astype(NPBF16) for b in range(B)]

    in_maps = []
    for c in range(N_CORES):
        b, hg = c // TP, c % TP
        qs, ks, vs = hg * QCOLS, D + hg * QCOLS, 2 * D + hg * QCOLS
        es = (c % TP) * ESL
        wqk = np.concatenate(
            [w_attn[:, qs:qs + QCOLS], w_attn[:, ks:ks + QCOLS]], axis=1
        ).astype(NPBF16)
        wv = np.ascontiguousarray(w_attn[:, vs:vs + QCOLS]).astype(NPBF16)
        # zg_sb chunk k=4*hp+j holds (heads 4j+2hp, 4j+2hp+1) x 64 dims;
        # permute w_proj rows to match the gathered layout.
        perm = np.empty(D, np.int64)
        for k in range(KC):
            hp_, j = k // TP, k % TP
            for p in range(128):
                perm[k * 128 + p] = (4 * j + 2 * hp_ + p // HD) * HD + p % HD
        wp = np.ascontiguousarray(w_proj[perm][:, es:es + ESL]).astype(NPBF16)
        bqk = np.stack([b_attn[qs:qs + 128], b_attn[qs + 128:qs + QCOLS],
                        b_attn[ks:ks + 128], b_attn[ks + 128:ks + QCOLS]],
                       axis=1).astype(np.float32)
        bv = np.ascontiguousarray(
            np.broadcast_to(b_attn[vs:vs + QCOLS], (128, QCOLS))).astype(np.float32)
        bp = np.ascontiguousarray(
            np.broadcast_to(b_proj[es:es + ESL], (128, ESL))).astype(np.float32)
        in_maps.append({
            "xT": xT[b], "wqk": wqk, "wv": wv, "wp": wp,
            "bqk": bqk, "bv": bv, "bp": bp, "tri": tri, "ones64": ones64,
        })
    return in_maps


def kernel(x, w_attn, b_attn, w_proj, b_proj):
    nc = _get_nc()
    in_maps = _make_in_maps(x, w_attn, b_attn, w_proj, b_proj)
    res = run_bass_kernel_spmd(nc, in_maps, core_ids=list(range(N_CORES)),
                               trace=bool(os.environ.get("BASS_TRACE")))
    if res.exec_time_ns is not None:
        print(f"HW exec time: {res.exec_time_ns} ns")
    out = np.empty((B, S, D), np.float32)
    for c in range(N_CORES):
        b, es = c // TP, (c % TP) * ESL
        out[b, :, es:es + ESL] = res.results[c]["out"]
    return out
